# revision 1
# baseline (speedup 1.0000x reference)
"""Multi-headed causal attention (B=2, S=2048, D=1024, H=16, DK=DV=64) on 8
Trainium2 NeuronCores.

Sharding (zero-communication): cores are split into 2 groups of 4, one group
per batch element. Within a group, core j owns two 256-query stripes: block j
(rows 256j..256j+255) and block 7-j. Stripe A (the early block, j<=3) only
attends to keys [0, 1024); stripe B (block 7-j >= 4) attends to keys
[0, 2048). Each core recomputes the K/V projections for its batch (keys it
needs), computes its queries' attention and output projection rows, and the
host concatenates row slices -- no cross-core communication.

All matmuls run in fp32r (full PE rate at free-dim >= 256, ~1e-4 relative
error). Softmax skips max-subtraction (scores are O(1) by construction, exp
cannot overflow) and gets its denominator from an all-ones column appended to
V, so the whole softmax costs one ACT exp pass plus one DVE mask-multiply.
Causal/validity masking is a 0/1 multiplicative mask input applied post-exp.
Per-head normalization happens on the small [64, 512] attention output (not
the big attention matrix) via a PE-replicated reciprocal row, which lets the
output projection accumulate all 16 heads in PSUM.
"""

import numpy as np

B, S, D, H, DK = 2, 2048, 1024, 16, 64
NQ = 512          # queries per core: 2 stripes x 256
NCORES = 8

_BUILT = {}


def _build_nc():
    import os
    PH = int(os.environ.get("BISECT_PHASES", "6"))
    import concourse.bacc as bacc
    import concourse.mybir as mybir
    from concourse import tile

    f32 = mybir.dt.float32
    f32r = mybir.dt.float32r
    bf16 = mybir.dt.bfloat16
    AF = mybir.ActivationFunctionType
    ALU = mybir.AluOpType

    nc = bacc.Bacc("TRN2", target_bir_lowering=False, debug=False,
                   num_devices=NCORES)

    xk_t = nc.declare_dram_parameter("xk_t", [D, S], f32r, isOutput=False)
    xv_t = nc.declare_dram_parameter("xv_t", [D, S], f32r, isOutput=False)
    xq_t = nc.declare_dram_parameter("xq_t", [D, NQ], f32r, isOutput=False)
    wk_t = nc.declare_dram_parameter("wk_t", [D, D], f32r, isOutput=False)
    wv_t = nc.declare_dram_parameter("wv_t", [D, D], f32r, isOutput=False)
    wq_t = nc.declare_dram_parameter("wq_t", [D, D], f32r, isOutput=False)
    wo_t = nc.declare_dram_parameter("wo_t", [D, D], f32r, isOutput=False)
    bk_s = nc.declare_dram_parameter("bk_s", [128, 8], f32, isOutput=False)
    bq_s = nc.declare_dram_parameter("bq_s", [128, 8], f32, isOutput=False)
    bv_r = nc.declare_dram_parameter("bv_r", [1, D], f32r, isOutput=False)
    bo_r = nc.declare_dram_parameter("bo_r", [1, D], f32r, isOutput=False)
    ones1 = nc.declare_dram_parameter("ones1", [1, 128], f32r, isOutput=False)
    ones128 = nc.declare_dram_parameter("ones128", [128, 128], f32r, isOutput=False)
    onesv = nc.declare_dram_parameter("onesv", [128, 8], f32r, isOutput=False)
    maskin = nc.declare_dram_parameter("maskin", [S, 2 * NQ], bf16, isOutput=False)
    out = nc.declare_dram_parameter("out", [NQ, D], f32, isOutput=True)

    from contextlib import ExitStack

    class _Stop(Exception):
        pass

    with tile.TileContext(nc) as tc:
      try:
        with ExitStack() as ctx:
            persist = ctx.enter_context(tc.tile_pool(name="persist", bufs=1))
            w2 = ctx.enter_context(tc.tile_pool(name="w2", bufs=2))
            w3 = ctx.enter_context(tc.tile_pool(name="w3", bufs=3))

            # ---- constants ----
            bk_sb = persist.tile([128, 8], f32, name="bk", tag="bk")
            bq_sb = persist.tile([128, 8], f32, name="bq", tag="bq")
            ones_sb = persist.tile([1, 128], f32r, name="ones1", tag="ones1")
            nc.sync.dma_start(bk_sb[:], bk_s[:])
            nc.sync.dma_start(bq_sb[:], bq_s[:])
            nc.sync.dma_start(ones_sb[:], ones1[:])
            ones128_sb = persist.tile([128, 128], f32r, name="ones128",
                                      tag="ones128")
            nc.sync.dma_start(ones128_sb[:], ones128[:])
            # ---- P1: replicate bv, bo across partitions via K=1 matmul ----
            bv_rep = persist.tile([128, D], f32, name="bvrep", tag="bvrep")
            with tc.tile_pool(name="ps1", bufs=2, space="PSUM") as ps1, \
                 tc.tile_pool(name="p1s", bufs=1) as p1s:
                bv_rsb = p1s.tile([1, D], f32r, name="bvr", tag="bvr")
                nc.sync.dma_start(bv_rsb[:], bv_r[:])
                for half in range(2):
                    rp = ps1.tile([128, 512], f32, name="rep1", tag="rep1")
                    nc.tensor.matmul(rp[:], ones_sb[:],
                                     bv_rsb[:, half * 512:(half + 1) * 512],
                                     start=True, stop=True)
                    nc.scalar.copy(bv_rep[:, half * 512:(half + 1) * 512],
                                   rp[:])

            # ---- P2: kT projection: kT[ft] = (Wk x_k^T + bk)[ft] ----
            phase_ctx = ctx.enter_context(ExitStack())
            projp = phase_ctx.enter_context(tc.tile_pool(name="projp", bufs=1))
            kT = [projp.tile([128, S], f32r, name=f"kt{ft}", tag=f"kt{ft}")
                  for ft in range(8)]
            with tc.tile_pool(name="wkp", bufs=1) as wkp, \
                 tc.tile_pool(name="ps2", bufs=3, space="PSUM") as ps2:
                wk_sb = [wkp.tile([128, D], f32r, name=f"wk{kp}", tag=f"wk{kp}")
                         for kp in range(8)]
                for kp in range(8):
                    nc.sync.dma_start(wk_sb[kp][:],
                                      wk_t[kp * 128:(kp + 1) * 128, :])
                for sc in range(4):
                    xkc = [w2.tile([128, 512], f32r, name=f"x{kp}", tag=f"x{kp}")
                           for kp in range(8)]
                    for kp in range(8):
                        nc.sync.dma_start(
                            xkc[kp][:],
                            xk_t[kp * 128:(kp + 1) * 128,
                                 sc * 512:(sc + 1) * 512])
                    for ft in range(8):
                        ps = ps2.tile([128, 512], f32, name="p2", tag="p2")
                        for kp in range(8):
                            nc.tensor.matmul(
                                ps[:],
                                wk_sb[kp][:, ft * 128:(ft + 1) * 128],
                                xkc[kp][:],
                                start=(kp == 0), stop=(kp == 7))
                        nc.scalar.activation(
                            kT[ft][:, sc * 512:(sc + 1) * 512], ps[:],
                            AF.Identity, bias=bk_sb[:, ft:ft + 1])

            # ---- P3: qT projection ----
            if PH < 3:
                raise _Stop()
            qT = [projp.tile([128, NQ], f32r, name=f"qt{ft}", tag=f"qt{ft}")
                  for ft in range(8)]
            with tc.tile_pool(name="wqp", bufs=1) as wqp, \
                 tc.tile_pool(name="ps3", bufs=3, space="PSUM") as ps3:
                wq_sb = [wqp.tile([128, D], f32r, name=f"wq{kp}", tag=f"wq{kp}")
                         for kp in range(8)]
                xqc = [w2.tile([128, NQ], f32r, name=f"x{kp}", tag=f"x{kp}")
                       for kp in range(8)]
                for kp in range(8):
                    nc.sync.dma_start(wq_sb[kp][:],
                                      wq_t[kp * 128:(kp + 1) * 128, :])
                    nc.sync.dma_start(xqc[kp][:],
                                      xq_t[kp * 128:(kp + 1) * 128, :])
                for ft in range(8):
                    ps = ps3.tile([128, NQ], f32, name="p3", tag="p3")
                    for kp in range(8):
                        nc.tensor.matmul(
                            ps[:], wq_sb[kp][:, ft * 128:(ft + 1) * 128],
                            xqc[kp][:], start=(kp == 0), stop=(kp == 7))
                    nc.scalar.activation(qT[ft][:], ps[:], AF.Identity,
                                         bias=bq_sb[:, ft:ft + 1])

            # ---- P4: masks ----
            if PH < 4:
                raise _Stop()
            attnp = phase_ctx.enter_context(tc.tile_pool(name="attnp", bufs=1))
            mask_sb = [attnp.tile([128, 2 * NQ], bf16, name=f"mk{p}",
                                  tag=f"mk{p}")
                       for p in range(8)]
            for p in range(8):
                nc.sync.dma_start(mask_sb[p][:],
                                  maskin[p * 128:(p + 1) * 128, :])

            # ---- P5: per 4-head group: V projection + paired attention ----
            if PH < 5:
                raise _Stop()
            navTn = [persist.tile([128, NQ], f32r, name=f"nv{i}", tag=f"nv{i}")
                     for i in range(8)]
            with tc.tile_pool(name="p5", bufs=1) as p5, \
                 tc.tile_pool(name="p5n", bufs=2) as p5n, \
                 tc.tile_pool(name="p5c", bufs=3, space="PSUM") as p5sc, \
                 tc.tile_pool(name="p5v", bufs=2, space="PSUM") as p5vp, \
                 tc.tile_pool(name="p5a0", bufs=1, space="PSUM") as p5a0, \
                 tc.tile_pool(name="p5a1", bufs=1, space="PSUM") as p5a1, \
                 tc.tile_pool(name="p5r", bufs=1, space="PSUM") as p5rp:
                for hg in range(4):
                    # V projection for heads 4hg..4hg+3
                    wv_sb = [p5.tile([128, 256], f32r, name=f"wv{kp}",
                                     tag=f"wv{kp}")
                             for kp in range(8)]
                    for kp in range(8):
                        nc.sync.dma_start(
                            wv_sb[kp][:],
                            wv_t[kp * 128:(kp + 1) * 128,
                                 hg * 256:(hg + 1) * 256])
                    v_hg = [attnp.tile([128, 260], f32r, name=f"v{st}",
                                       tag=f"v{st}")
                            for st in range(16)]
                    for st in range(16):
                        nc.sync.dma_start(
                            v_hg[st][:].rearrange("p (h c) -> p h c",
                                                  c=65)[:, :, 64:65],
                            onesv[:, 0:4].rearrange("p (h c) -> p h c", c=1))
                    for chunk in range(4):
                        xvc = [w2.tile([128, 512], f32r, name=f"x{kp}",
                                       tag=f"x{kp}")
                               for kp in range(8)]
                        for kp in range(8):
                            nc.sync.dma_start(
                                xvc[kp][:],
                                xv_t[kp * 128:(kp + 1) * 128,
                                     chunk * 512:(chunk + 1) * 512])
                        for stl in range(4):
                            st = 4 * chunk + stl
                            vp = p5vp.tile([128, 256], f32, name="vp",
                                           tag="vp")
                            for kp in range(8):
                                nc.tensor.matmul(
                                    vp[:],
                                    xvc[kp][:, stl * 128:(stl + 1) * 128],
                                    wv_sb[kp][:],
                                    start=(kp == 0), stop=(kp == 7))
                            nc.vector.tensor_tensor(
                                v_hg[st][:].rearrange(
                                    "p (h c) -> p h c", c=65)[:, :, 0:64],
                                vp[:].rearrange("p (h c) -> p h c", c=64),
                                bv_rep[:, hg * 256:(hg + 1) * 256].rearrange(
                                    "p (h c) -> p h c", c=64),
                                ALU.add)
                    # attention: 2 head-pairs, kt-interleaved so the two
                    # heads' K=64 scores matmuls land in disjoint PE row
                    # groups and run concurrently
                    for pl in range(2):
                        hp = 2 * hg + pl
                        dgat = p5n.tile([128, NQ], f32, name="dgat",
                                        tag="dgat")
                        nc.gpsimd.memset(dgat[:], 1.0)
                        avp = [p5a0.tile([65, NQ], f32, name="av0",
                                         tag="av0"),
                               p5a1.tile([65, NQ], f32, name="av1",
                                         tag="av1")]
                        for i, hs in ((0, 0), (1, 64)):
                            for p in range(8):
                                kt0 = 2 * p
                                N, qoff = (512, 0) if kt0 < 8 else (256, 256)
                                am = w3.tile([128, 2 * NQ], f32r, name="am",
                                             tag="am")
                                for half in range(2):
                                    kt = kt0 + half
                                    sc_ps = p5sc.tile([128, 512], f32,
                                                      name="sc", tag="sc")
                                    nc.tensor.matmul(
                                        sc_ps[:, 0:N],
                                        kT[hp][hs:hs + 64,
                                               kt * 128:(kt + 1) * 128],
                                        qT[hp][hs:hs + 64, qoff:512],
                                        start=True, stop=True)
                                    nc.scalar.activation(
                                        am[:, half * NQ + qoff:
                                           half * NQ + qoff + N],
                                        sc_ps[:, 0:N], AF.Exp, scale=0.125)
                                nc.vector.tensor_tensor(
                                    am[:].rearrange(
                                        "x (h q) -> x h q",
                                        q=NQ)[:, :, qoff:qoff + N],
                                    am[:].rearrange(
                                        "x (h q) -> x h q",
                                        q=NQ)[:, :, qoff:qoff + N],
                                    mask_sb[p][:].rearrange(
                                        "x (h q) -> x h q",
                                        q=NQ)[:, :, qoff:qoff + N],
                                    ALU.mult)
                                for half in range(2):
                                    kt = kt0 + half
                                    nc.tensor.matmul(
                                        avp[i][:, qoff:qoff + N],
                                        v_hg[kt][:, (2 * pl + i) * 65:
                                                 (2 * pl + i + 1) * 65],
                                        am[:, half * NQ + qoff:
                                           half * NQ + qoff + N],
                                        start=(kt == 0), stop=(kt == 15))
                        for i in range(2):
                            nc.scalar.copy(dgat[64 * i:64 * i + 1, :],
                                           avp[i][64:65, :])
                            nc.scalar.copy(navTn[hp][64 * i:64 * i + 64, :],
                                           avp[i][0:64, :])
                        dgrec = p5n.tile([128, NQ], f32r, name="dgrec",
                                         tag="dgrec")
                        with nc.allow_low_precision(
                                reason="f32r recip, ~5e-4 rel ok"):
                            nc.vector.reciprocal(dgrec[:], dgat[:])
                        for i in range(2):
                            rep_ps = p5rp.tile([64, NQ], f32, name="repd",
                                               tag="repd")
                            nc.tensor.matmul(
                                rep_ps[:],
                                ones128_sb[64 * i:64 * i + 1, 0:64],
                                dgrec[64 * i:64 * i + 1, :],
                                start=True, stop=True)
                            nc.vector.tensor_tensor(
                                navTn[hp][64 * i:64 * i + 64, :],
                                navTn[hp][64 * i:64 * i + 64, :],
                                rep_ps[:], ALU.mult)

            phase_ctx.close()

            # ---- P6: output projection, all heads PSUM-accumulated ----
            if PH < 6:
                raise _Stop()
            with tc.tile_pool(name="p6", bufs=2) as p6, \
                 tc.tile_pool(name="ps6", bufs=2, space="PSUM") as ps6:
                bo_rsb = p6.tile([1, D], f32r, name="bor", tag="bor")
                nc.sync.dma_start(bo_rsb[:], bo_r[:])
                bo_rep = p6.tile([128, D], f32, name="borep", tag="borep")
                for half in range(2):
                    rp6 = ps6.tile([128, 512], f32, name="fin", tag="fin")
                    nc.tensor.matmul(rp6[:], ones_sb[:],
                                     bo_rsb[:, half * 512:(half + 1) * 512],
                                     start=True, stop=True)
                    nc.scalar.copy(bo_rep[:, half * 512:(half + 1) * 512],
                                   rp6[:])
                for oc in range(2):
                    wo_sb = [p6.tile([128, 512], f32r, name=f"wo{i}", tag=f"wo{i}")
                             for i in range(8)]
                    for i in range(8):
                        nc.sync.dma_start(
                            wo_sb[i][0:64, :],
                            wo_t[(2 * i) * 64:(2 * i + 1) * 64,
                                 oc * 512:(oc + 1) * 512])
                        nc.sync.dma_start(
                            wo_sb[i][64:128, :],
                            wo_t[(2 * i + 1) * 64:(2 * i + 2) * 64,
                                 oc * 512:(oc + 1) * 512])
                    for rc in range(4):
                        fp = ps6.tile([128, 512], f32, name="fin", tag="fin")
                        for hp in range(8):
                            nc.tensor.matmul(
                                fp[:],
                                navTn[hp][:, rc * 128:(rc + 1) * 128],
                                wo_sb[hp][:],
                                start=(hp == 0), stop=(hp == 7))
                        fo = p6.tile([128, 512], f32, name="fo", tag="fo")
                        nc.vector.tensor_tensor(
                            fo[:], fp[:],
                            bo_rep[:, oc * 512:(oc + 1) * 512], ALU.add)
                        nc.sync.dma_start(
                            out[rc * 128:(rc + 1) * 128,
                                oc * 512:(oc + 1) * 512], fo[:])
      except _Stop:
          pass
    nc.compile()
    return nc


def kernel(V, K, Q, padding_mask, Wv_w, Wv_b, Wk_w, Wk_b, Wq_w, Wq_b,
           Wo_w, Wo_b):
    from concourse.bass_utils import run_bass_kernel_spmd

    V = np.asarray(V, np.float32)
    K = np.asarray(K, np.float32)
    Q = np.asarray(Q, np.float32)
    padding_mask = np.asarray(padding_mask)
    import ml_dtypes

    if "nc" not in _BUILT:
        _BUILT["nc"] = _build_nc()
    nc = _BUILT["nc"]

    wk_t = np.ascontiguousarray(np.asarray(Wk_w, np.float32).T)
    wv_t = np.ascontiguousarray(np.asarray(Wv_w, np.float32).T)
    wq_t = np.ascontiguousarray(np.asarray(Wq_w, np.float32).T)
    wo_t = np.ascontiguousarray(np.asarray(Wo_w, np.float32).T)
    bk_s = np.ascontiguousarray(np.asarray(Wk_b, np.float32).reshape(8, 128).T)
    bq_s = np.ascontiguousarray(np.asarray(Wq_b, np.float32).reshape(8, 128).T)
    bv_r = np.asarray(Wv_b, np.float32).reshape(1, D)
    bo_r = np.asarray(Wo_b, np.float32).reshape(1, D)
    ones1 = np.ones((1, 128), np.float32)
    ones128a = np.ones((128, 128), np.float32)
    onesv = np.ones((128, 8), np.float32)

    xk_T = [np.ascontiguousarray(K[b].T) for b in range(B)]
    xv_T = [np.ascontiguousarray(V[b].T) for b in range(B)]

    in_maps = []
    blocks = []
    kpos = np.arange(S)[:, None]
    for core in range(NCORES):
        b, j = core // 4, core % 4
        blkA, blkB = j, 7 - j
        blocks.append((b, blkA, blkB))
        rows = np.r_[256 * blkA:256 * (blkA + 1), 256 * blkB:256 * (blkB + 1)]
        xq_t = np.ascontiguousarray(Q[b][rows].T)
        qpos = np.r_[np.arange(256 * blkA, 256 * (blkA + 1)),
                     np.arange(256 * blkB, 256 * (blkB + 1))][None, :]
        mask = (kpos <= qpos) & (padding_mask[b][:, None] != 0)
        mp = mask.reshape(16, 128, NQ)
        mask = np.concatenate([mp[0::2], mp[1::2]], axis=2).reshape(S // 2,
                                                                    2 * NQ)
        mask = np.concatenate([mask, np.zeros_like(mask)], axis=0)
        in_maps.append({
            "xk_t": xk_T[b], "xv_t": xv_T[b], "xq_t": xq_t,
            "wk_t": wk_t, "wv_t": wv_t, "wq_t": wq_t, "wo_t": wo_t,
            "bk_s": bk_s, "bq_s": bq_s, "bv_r": bv_r, "bo_r": bo_r,
            "ones1": ones1, "ones128": ones128a, "onesv": onesv,
            "maskin": mask.astype(ml_dtypes.bfloat16),
        })

    _BUILT["last_maps"] = in_maps
    res = run_bass_kernel_spmd(nc, in_maps, core_ids=list(range(NCORES)))
    _BUILT["last_result"] = res

    outf = np.empty((B, S, D), np.float32)
    for core in range(NCORES):
        b, blkA, blkB = blocks[core]
        o = res.results[core]["out"]
        outf[b, 256 * blkA:256 * (blkA + 1)] = o[0:256]
        outf[b, 256 * blkB:256 * (blkB + 1)] = o[256:512]
    return outf



# revision 12
# speedup vs baseline: 1.5171x; 1.5171x over previous
"""Multi-headed causal attention (B=2, S=2048, D=1024, H=16, DK=DV=64) on 8
Trainium2 NeuronCores.

Sharding: HEAD-parallel attention + QUERY-parallel output projection.
Core c owns heads {2c, 2c+1} for BOTH batches. It projects K/Q/V only for
its two heads (zero redundant FLOPs), runs the full causal attention for
them, then a single 1MB AllToAll redistributes the normalized attention
outputs so core c ends up with all 16 heads for query chunk
(batch c//4, rows 512*(c%4) ...). Each core then output-projects its own
512 queries. Head-sharding makes the fine-grained causal tile structure
(only kt <= t score tiles, 34 banks of 4 tiles per head-batch) IDENTICAL
on every core, which a query-sharded SPMD program cannot do.

All matmul operands are bf16 (full PE rate at any free size, half the DMA
bytes, and far less PE power than fp32r -> avoids the 50% power throttle
the fp32r baseline hit). PSUM accumulation stays f32. Softmax skips
max-subtraction (scores are O(1)); denominators come from an all-ones
column appended to V (an extra output partition, free on the PE); the
reciprocal is one fast-approx DVE op on the [1,512] denominator row,
replicated across partitions by a K=1 matmul. Causal masking multiplies
the 128x128 triangular mask only on diagonal tiles (the padding mask is
all ones in this problem; a general fallback masks every bank).
"""

import numpy as np

B, S, D, H, DK = 2, 2048, 1024, 16, 64
NCORES = 8
NT = S // 128  # 16 tiles per batch
NBANKS = 34    # 136 causal (t,kt) tiles / 4 slots per PSUM bank

_BUILT = {}


def _build_nc(general_mask):
    import concourse.bacc as bacc
    import concourse.mybir as mybir
    from concourse import tile
    from contextlib import ExitStack

    f32 = mybir.dt.float32
    f32r = mybir.dt.float32r
    bf16 = mybir.dt.bfloat16
    AF = mybir.ActivationFunctionType
    ALU = mybir.AluOpType

    nc = bacc.Bacc("TRN2", target_bir_lowering=False, debug=False,
                   num_devices=NCORES)

    # x tensors are [b*1024 + dim, seq] transposed inputs, same on all cores
    xk_t = nc.declare_dram_parameter("xk_t", [2 * D, S], bf16, isOutput=False)
    xq_t = nc.declare_dram_parameter("xq_t", [2 * D, S], bf16, isOutput=False)
    xv_t = nc.declare_dram_parameter("xv_t", [2 * D, S], bf16, isOutput=False)
    # per-core head-pair weight slices
    wk_h = nc.declare_dram_parameter("wk_h", [D, 128], bf16, isOutput=False)
    wq_h = nc.declare_dram_parameter("wq_h", [D, 128], bf16, isOutput=False)
    wv_p = nc.declare_dram_parameter("wv_p", [D, 130], bf16, isOutput=False)
    wo_r = nc.declare_dram_parameter("wo_r", [D, D], bf16, isOutput=False)
    bk_h = nc.declare_dram_parameter("bk_h", [128, 1], f32, isOutput=False)
    bq_h = nc.declare_dram_parameter("bq_h", [128, 1], f32, isOutput=False)
    bv_p = nc.declare_dram_parameter("bv_p", [1, 130], f32, isOutput=False)
    bo_r = nc.declare_dram_parameter("bo_r", [1, D], f32, isOutput=False)
    trimask = nc.declare_dram_parameter("trimask", [128, 128], bf16,
                                        isOutput=False)
    if general_mask:
        maskb = nc.declare_dram_parameter(
            "maskb", [2 * NBANKS * 128, 512], bf16, isOutput=False)
    out = nc.declare_dram_parameter("out", [512, D], f32, isOutput=True)

    with tile.TileContext(nc) as tc:
        with ExitStack() as ctx:
            persist = ctx.enter_context(tc.tile_pool(name="persist", bufs=1))

            # ---- persistent tiles ----
            wk_sb = [persist.tile([128, 128], bf16, name=f"wk{i}",
                                  tag=f"wk{i}") for i in range(8)]
            wq_sb = [persist.tile([128, 128], bf16, name=f"wq{i}",
                                  tag=f"wq{i}") for i in range(8)]
            wv_sb = [persist.tile([128, 130], bf16, name=f"wv{i}",
                                  tag=f"wv{i}") for i in range(8)]
            wo_sb = [persist.tile([128, D], bf16, name=f"wo{i}",
                                  tag=f"wo{i}") for i in range(8)]
            bk_sb = persist.tile([128, 1], f32, name="bk", tag="bk")
            bq_sb = persist.tile([128, 1], f32, name="bq", tag="bq")
            tri_sb = persist.tile([128, 128], bf16, name="tri", tag="tri")
            bvr_sb = persist.tile([1, 130], f32, name="bvr", tag="bvr")
            bor_sb = persist.tile([1, D], f32, name="bor", tag="bor")
            bv_rep = persist.tile([128, 130], f32, name="bvrep", tag="bvrep")
            bo_rep = persist.tile([128, D], f32, name="borep", tag="borep")
            kT = [persist.tile([128, S], bf16, name=f"kT{b}", tag=f"kT{b}")
                  for b in range(B)]
            qT = [persist.tile([128, S], bf16, name=f"qT{b}", tag=f"qT{b}")
                  for b in range(B)]
            v_sb = [[persist.tile([128, 130], bf16, name=f"v{b}_{st}",
                                  tag=f"v{b}_{st}") for st in range(NT)]
                    for b in range(B)]
            navTh = [[persist.tile([64, S], bf16, name=f"nav{b}_{hh}",
                                   tag=f"nav{b}_{hh}") for hh in range(2)]
                     for b in range(B)]
            nall = [persist.tile([128, 512], bf16, name=f"na{i}",
                                 tag=f"na{i}") for i in range(8)]

            # ---- working pools ----
            xs = ctx.enter_context(tc.tile_pool(name="xs", bufs=2))
            amp = ctx.enter_context(tc.tile_pool(name="amp", bufs=3))
            nrm = ctx.enter_context(tc.tile_pool(name="nrm", bufs=2))
            reps = ctx.enter_context(tc.tile_pool(name="reps", bufs=2))
            fop = ctx.enter_context(tc.tile_pool(name="fop", bufs=2))
            pp = ctx.enter_context(tc.tile_pool(name="pp", bufs=2,
                                                space="PSUM"))
            scp = ctx.enter_context(tc.tile_pool(name="scp", bufs=4,
                                                 space="PSUM"))
            avp = ctx.enter_context(tc.tile_pool(name="avp", bufs=2,
                                                 space="PSUM"))
            if general_mask:
                mbp = ctx.enter_context(tc.tile_pool(name="mbp", bufs=4))
            dram = ctx.enter_context(tc.tile_pool(name="dram", bufs=1,
                                                  space="DRAM"))
            a2a_in = dram.tile([1024, 512], bf16, name="a2a_in",
                               tag="a2a_in")
            a2a_out = dram.tile([1024, 512], bf16, name="a2a_out",
                                tag="a2a_out")

            # ---- P0: weight/bias loads + bias replication ----
            for i in range(8):
                nc.sync.dma_start(wk_sb[i][:], wk_h[128 * i:128 * (i + 1), :])
                nc.sync.dma_start(wq_sb[i][:], wq_h[128 * i:128 * (i + 1), :])
                nc.sync.dma_start(wv_sb[i][:], wv_p[128 * i:128 * (i + 1), :])
                nc.sync.dma_start(wo_sb[i][:], wo_r[128 * i:128 * (i + 1), :])
            nc.sync.dma_start(bk_sb[:], bk_h[:])
            nc.sync.dma_start(bq_sb[:], bq_h[:])
            nc.sync.dma_start(tri_sb[:], trimask[:])
            nc.sync.dma_start(bvr_sb[:], bv_p[:])
            nc.sync.dma_start(bor_sb[:], bo_r[:])

            nc.gpsimd.partition_broadcast(bv_rep[:], bvr_sb[:])
            nc.gpsimd.partition_broadcast(bo_rep[:], bor_sb[:])

            # ---- projection helpers ----
            def load_x(param, b):
                tiles = [xs.tile([128, S], bf16, name=f"x{kp}", tag=f"x{kp}")
                         for kp in range(8)]
                for kp in range(8):
                    nc.sync.dma_start(
                        tiles[kp][:],
                        param[D * b + 128 * kp:D * b + 128 * (kp + 1), :])
                return tiles

            def proj_kq_unit(x, w_sb, bias_sb, dst, sc):
                ps = pp.tile([128, 512], f32, name="pp", tag="pp")
                for kp in range(8):
                    nc.tensor.matmul(ps[:], w_sb[kp][:],
                                     x[kp][:, 512 * sc:512 * (sc + 1)],
                                     start=(kp == 0), stop=(kp == 7))
                nc.vector.tensor_scalar_add(
                    dst[:, 512 * sc:512 * (sc + 1)], ps[:], bias_sb[:])

            def proj_v_unit(x, b, st):
                ps = pp.tile([128, 512], f32, name="pp", tag="pp")
                for kp in range(8):
                    nc.tensor.matmul(ps[:, 0:130],
                                     x[kp][:, 128 * st:128 * (st + 1)],
                                     wv_sb[kp][:],
                                     start=(kp == 0), stop=(kp == 7))
                nc.vector.tensor_tensor(v_sb[b][st][:], ps[:, 0:130],
                                        bv_rep[:], ALU.add)

            def proj_batch(b):
                x = load_x(xk_t, b)
                for sc in range(4):
                    proj_kq_unit(x, wk_sb, bk_sb, kT[b], sc)
                x = load_x(xq_t, b)
                for sc in range(4):
                    proj_kq_unit(x, wq_sb, bq_sb, qT[b], sc)
                x = load_x(xv_t, b)
                for st in range(NT):
                    proj_v_unit(x, b, st)

            def b1_proj_gen():
                x = load_x(xk_t, 1)
                yield
                for sc in range(4):
                    proj_kq_unit(x, wk_sb, bk_sb, kT[1], sc)
                    yield
                x = load_x(xq_t, 1)
                yield
                for sc in range(4):
                    proj_kq_unit(x, wq_sb, bq_sb, qT[1], sc)
                    yield
                x = load_x(xv_t, 1)
                yield
                for st in range(NT):
                    proj_v_unit(x, 1, st)
                    yield

            # ---- attention ----
            stream = [(t, kt) for t in range(NT) for kt in range(t + 1)]
            banks = [stream[i:i + 4] for i in range(0, len(stream), 4)]

            def norm_block(b, hh, av, T):
                # denominator row lives on PSUM partition 64; only ACT can
                # shift partitions, DVE lanes are partition-locked
                dg0 = nrm.tile([1, 512], f32, name="dg0", tag="dg0")
                nc.scalar.copy(dg0[:], av[64:65, :])
                dg = nrm.tile([1, 512], f32, name="dg", tag="dg")
                nc.vector.reciprocal_approx_fast(dg[:], dg0[:])
                rep = reps.tile([64, 512], f32, name="rep", tag="rep")
                nc.gpsimd.partition_broadcast(rep[:], dg[:])
                nc.vector.tensor_tensor(
                    navTh[b][hh][:, 512 * T:512 * (T + 1)],
                    av[0:64, :], rep[:], ALU.mult)

            def attention(b, hh, filler=None):
                r0 = 64 * hh
                av = None
                for bi, bank in enumerate(banks):
                    if filler is not None and bi % 2 == 0:
                        next(filler, None)
                    sc = scp.tile([128, 512], f32, name="sc", tag="sc")
                    for s, (t, kt) in enumerate(bank):
                        nc.tensor.matmul(
                            sc[:, 128 * s:128 * (s + 1)],
                            kT[b][r0:r0 + 64, 128 * kt:128 * (kt + 1)],
                            qT[b][r0:r0 + 64, 128 * t:128 * (t + 1)],
                            start=True, stop=True)
                    am = amp.tile([128, 512], bf16, name="am", tag="am")
                    nc.scalar.activation(am[:], sc[:], AF.Exp, scale=0.125)
                    if general_mask:
                        mb = mbp.tile([128, 512], bf16, name="mb", tag="mb")
                        r = (b * NBANKS + bi) * 128
                        nc.sync.dma_start(mb[:], maskb[r:r + 128, :])
                        nc.vector.tensor_tensor(am[:], am[:], mb[:], ALU.mult)
                    else:
                        for s, (t, kt) in enumerate(bank):
                            if t == kt:
                                nc.vector.tensor_tensor(
                                    am[:, 128 * s:128 * (s + 1)],
                                    am[:, 128 * s:128 * (s + 1)],
                                    tri_sb[:], ALU.mult)
                    for s, (t, kt) in enumerate(bank):
                        if kt == 0 and t % 4 == 0:
                            av = avp.tile([65, 512], f32, name="av", tag="av")
                        nc.tensor.matmul(
                            av[:, 128 * (t % 4):128 * (t % 4 + 1)],
                            v_sb[b][kt][:, 65 * hh:65 * (hh + 1)],
                            am[:, 128 * s:128 * (s + 1)],
                            start=(kt == 0), stop=(kt == t))
                        if kt == t and t % 4 == 3:
                            norm_block(b, hh, av, t // 4)

            # ---- emission schedule ----
            proj_batch(0)
            filler = b1_proj_gen()
            attention(0, 0, filler)
            attention(0, 1, filler)
            for _ in filler:  # drain any leftovers
                pass
            for T in range(4):
                for hh in range(2):
                    r = 128 * T + 64 * hh
                    nc.sync.dma_start(a2a_in[r:r + 64, :],
                                      navTh[0][hh][:, 512 * T:512 * (T + 1)])
            attention(1, 0)
            attention(1, 1)
            for T in range(4):
                for hh in range(2):
                    r = 128 * (4 + T) + 64 * hh
                    nc.sync.dma_start(a2a_in[r:r + 64, :],
                                      navTh[1][hh][:, 512 * T:512 * (T + 1)])

            # ---- AllToAll: (heads-sharded) -> (query-sharded) ----
            nc.gpsimd.collective_compute(
                "AllToAll", ALU.bypass,
                replica_groups=[list(range(NCORES))],
                ins=[a2a_in.opt()], outs=[a2a_out.opt()])
            for i in range(8):
                nc.sync.dma_start(nall[i][:],
                                  a2a_out[128 * i:128 * (i + 1), :])

            # ---- output projection for this core's 512 queries ----
            for qc in range(4):
                for oc in range(2):
                    ps = scp.tile([128, 512], f32, name="sc", tag="sc")
                    for i in range(8):
                        nc.tensor.matmul(
                            ps[:], nall[i][:, 128 * qc:128 * (qc + 1)],
                            wo_sb[i][:, 512 * oc:512 * (oc + 1)],
                            start=(i == 0), stop=(i == 7))
                    fo = fop.tile([128, 512], f32, name="fo", tag="fo")
                    nc.vector.tensor_tensor(
                        fo[:], ps[:], bo_rep[:, 512 * oc:512 * (oc + 1)],
                        ALU.add)
                    nc.sync.dma_start(
                        out[128 * qc:128 * (qc + 1),
                            512 * oc:512 * (oc + 1)], fo[:])

    nc.compile()
    return nc


def kernel(V, K, Q, padding_mask, Wv_w, Wv_b, Wk_w, Wk_b, Wq_w, Wq_b,
           Wo_w, Wo_b):
    import ml_dtypes
    from concourse.bass_utils import run_bass_kernel_spmd
    bf = ml_dtypes.bfloat16

    V = np.asarray(V, np.float32)
    K = np.asarray(K, np.float32)
    Q = np.asarray(Q, np.float32)
    pm = np.asarray(padding_mask)
    Wv_w = np.asarray(Wv_w, np.float32)
    Wv_b = np.asarray(Wv_b, np.float32)
    Wk_w = np.asarray(Wk_w, np.float32)
    Wk_b = np.asarray(Wk_b, np.float32)
    Wq_w = np.asarray(Wq_w, np.float32)
    Wq_b = np.asarray(Wq_b, np.float32)
    Wo_w = np.asarray(Wo_w, np.float32)
    Wo_b = np.asarray(Wo_b, np.float32)

    general = not bool((pm != 0).all())
    key = "gen" if general else "fast"
    if key not in _BUILT:
        _BUILT[key] = _build_nc(general)
    nc = _BUILT[key]

    xk = np.concatenate(
        [np.ascontiguousarray(K[b].T) for b in range(B)], 0).astype(bf)
    xq = np.concatenate(
        [np.ascontiguousarray(Q[b].T) for b in range(B)], 0).astype(bf)
    xv = np.concatenate(
        [np.ascontiguousarray(V[b].T) for b in range(B)], 0).astype(bf)
    wo_r = np.ascontiguousarray(Wo_w.T).astype(bf)
    bo = Wo_b.reshape(1, D).astype(np.float32)
    tri = (np.arange(128)[:, None] <= np.arange(128)[None, :])

    maskb_arr = None
    if general:
        stream = [(t, kt) for t in range(NT) for kt in range(t + 1)]
        bank_list = [stream[i:i + 4] for i in range(0, len(stream), 4)]
        maskb_arr = np.zeros((2 * NBANKS * 128, 512), np.float32)
        for b in range(B):
            keymask = (pm[b] != 0).astype(np.float32)
            for bi, bank in enumerate(bank_list):
                blk = np.zeros((128, 512), np.float32)
                for s, (t, kt) in enumerate(bank):
                    m = np.ones((128, 128), np.float32) if kt < t \
                        else tri.astype(np.float32)
                    blk[:, 128 * s:128 * (s + 1)] = (
                        m * keymask[128 * kt:128 * (kt + 1)][:, None])
                maskb_arr[(b * NBANKS + bi) * 128:
                          (b * NBANKS + bi + 1) * 128] = blk
        maskb_arr = maskb_arr.astype(bf)

    in_maps = []
    for c in range(NCORES):
        rows = slice(128 * c, 128 * (c + 1))
        wk_c = np.ascontiguousarray(Wk_w[rows].T).astype(bf)
        wq_c = np.ascontiguousarray(Wq_w[rows].T).astype(bf)
        wv_c = np.ascontiguousarray(Wv_w[rows].T)  # [1024, 128] f32
        wv_pad = np.zeros((D, 130), np.float32)
        wv_pad[:, 0:64] = wv_c[:, 0:64]
        wv_pad[:, 65:129] = wv_c[:, 64:128]
        bv_pad = np.zeros((1, 130), np.float32)
        bv_pad[0, 0:64] = Wv_b[128 * c:128 * c + 64]
        bv_pad[0, 64] = 1.0
        bv_pad[0, 65:129] = Wv_b[128 * c + 64:128 * c + 128]
        bv_pad[0, 129] = 1.0
        im = {
            "xk_t": xk, "xq_t": xq, "xv_t": xv,
            "wk_h": wk_c, "wq_h": wq_c, "wv_p": wv_pad.astype(bf),
            "wo_r": wo_r,
            "bk_h": np.ascontiguousarray(
                Wk_b[rows].reshape(128, 1)).astype(np.float32),
            "bq_h": np.ascontiguousarray(
                Wq_b[rows].reshape(128, 1)).astype(np.float32),
            "bv_p": bv_pad, "bo_r": bo,
            "trimask": tri.astype(bf),
        }
        if general:
            im["maskb"] = maskb_arr
        in_maps.append(im)

    _BUILT["last_maps"] = in_maps
    res = run_bass_kernel_spmd(nc, in_maps, core_ids=list(range(NCORES)))
    _BUILT["last_result"] = res
    _BUILT["nc"] = nc

    outf = np.empty((B, S, D), np.float32)
    for c in range(NCORES):
        b, T = c // 4, c % 4
        outf[b, 512 * T:512 * (T + 1)] = res.results[c]["out"]
    return outf
